# revision 3
# baseline (speedup 1.0000x reference)
"""KNN graph kernel for Trainium2 (8 NeuronCores, SPMD), single launch.

Per core (2500 query rows of 20000, padded to 2560 = 20 tiles of 128):
  scores s[q,j] = x_q . x_j - ||x_j||^2/2 (same ranking as -d2), computed as
  bf16 h/l split: h_q.h_j + h_q.l_j + l_q.h_j (+ 3-way bf16 split of the norm
  term via a K=3 ones matmul), accumulated in fp32 PSUM in 2048-col units
  (2 units double-buffered = the whole 16KB/partition PSUM).
  DVE scans run straight from PSUM (max8 + find_index8 per unit): removing
  the v1 ACT staging copy eliminated all DVE SBUF traffic, which measured as
  a 25% scan slowdown (2738ns vs the 2192ns hw rate for 2048-elem scans).
  Offline-validated: top-8-per-2048 winnow + exact fp32 ranking gives
  rel_err 7.5e-3 vs the jax reference (38/320000 elems).
  L2 merge (top-17 of the 80 pooled winners, 3 rounds of max8/max_index/
  match_replace) is emitted one op at a time into the MAX8->FIND_INDEX8
  dependency-drain windows of later tiles' scans; index extraction runs on
  the otherwise-idle ACT engine: d_s = Abs(C - 65536*Pu_s) via activation
  with per-partition bias (C[p,i] = 65536*i + gidx[p,i], fp32-exact), then
  one DVE tensor_reduce(min,|.|) -> exactly gidx[p, Pu[p,s]] (matching slot
  < 20480, any other slot >= 45056). 48 dummy ones3 matmuls during the
  input-DMA window warm the PE out of its 0.65GHz cold p-state.
"""
import numpy as np
import ml_dtypes

N, D, KOUT = 20000, 128, 16
NCORES = 8
RPC = 2500           # real rows per core
P = 128              # partitions / rows per tile
NTILES = 20          # row tiles per core (2560 rows incl. 60 pad)
UNIT = 2048          # scan unit
NPS = 2              # PSUM tiles in flight (UNIT*NPS*4B <= 16KB/partition)
SEC = 512            # matmul moving width
NPAD = 20480         # padded db columns
NU = NPAD // UNIT    # units per tile
POOL = NU * 8        # pooled winners per row
NEG = -6.0e4         # pad score, far below any real score (min real ~ -400)
NSEL = KOUT          # ranks 1..16 extracted (rank 0 = self dropped)

_compiled = None


def _split3_bf16(v32: np.ndarray) -> np.ndarray:
    h = v32.astype(ml_dtypes.bfloat16)
    r1 = v32 - h.astype(np.float32)
    m = r1.astype(ml_dtypes.bfloat16)
    r2 = r1 - m.astype(np.float32)
    l = r2.astype(ml_dtypes.bfloat16)
    return np.stack([h, m, l], axis=0)


def build_program(n_tiles=NTILES):
    import concourse.mybir as mybir
    import concourse.tile as tile
    from concourse import bacc

    nc = bacc.Bacc("TRN2", target_bir_lowering=False, debug=False, num_devices=NCORES)

    bf16 = mybir.dt.bfloat16
    f32 = mybir.dt.float32
    u16 = mybir.dt.uint16
    NSEC = 8
    SW = NPAD // NSEC
    xh_d = [nc.dram_tensor(f"xh{s}", [D, SW], bf16, kind="ExternalInput").ap()
            for s in range(NSEC)]
    xl_d = [nc.dram_tensor(f"xl{s}", [D, SW], bf16, kind="ExternalInput").ap()
            for s in range(NSEC)]
    qh0_d = nc.dram_tensor("qh0", [D, P], bf16, kind="ExternalInput").ap()
    ql0_d = nc.dram_tensor("ql0", [D, P], bf16, kind="ExternalInput").ap()
    qhr_d = nc.dram_tensor("qhr", [D, (n_tiles - 1) * P], bf16, kind="ExternalInput").ap()
    qlr_d = nc.dram_tensor("qlr", [D, (n_tiles - 1) * P], bf16, kind="ExternalInput").ap()
    nb3_d = nc.dram_tensor("nb3", [3, NPAD], bf16, kind="ExternalInput").ap()
    cio_d = nc.dram_tensor("cio", [P, POOL], f32, kind="ExternalInput").ap()
    out_d = nc.dram_tensor("out", [n_tiles * P, KOUT], mybir.dt.int32, kind="ExternalOutput").ap()

    with tile.TileContext(nc) as tc:
        with tc.tile_pool(name="const", bufs=1) as cpool, \
             tc.tile_pool(name="work", bufs=4) as wpool, \
             tc.tile_pool(name="ps", bufs=NPS, space="PSUM") as ppool:
            xh = [cpool.tile([D, SW], bf16, name=f"xh{s}", tag=f"xh{s}")
                  for s in range(NSEC)]
            xl = [cpool.tile([D, SW], bf16, name=f"xl{s}", tag=f"xl{s}")
                  for s in range(NSEC)]
            qh0 = cpool.tile([D, P], bf16, tag="qh0")
            ql0 = cpool.tile([D, P], bf16, tag="ql0")
            qhr = cpool.tile([D, (n_tiles - 1) * P], bf16, tag="qhr")
            qlr = cpool.tile([D, (n_tiles - 1) * P], bf16, tag="qlr")
            nb3 = cpool.tile([3, NPAD], bf16, tag="nb3")
            ones3 = cpool.tile([3, P], bf16, tag="ones3")
            cio = cpool.tile([P, POOL], f32, tag="cio")
            # DMA issue order = first-needed first
            nc.sync.dma_start(xh[0], xh_d[0])
            nc.sync.dma_start(xl[0], xl_d[0])
            nc.sync.dma_start(qh0, qh0_d)
            nc.sync.dma_start(ql0, ql0_d)
            nc.sync.dma_start(nb3, nb3_d)
            nc.sync.dma_start(xh[1], xh_d[1])
            nc.sync.dma_start(xl[1], xl_d[1])
            nc.sync.dma_start(qhr, qhr_d)
            nc.sync.dma_start(qlr, qlr_d)
            for s in range(2, NSEC):
                nc.sync.dma_start(xh[s], xh_d[s])
                nc.sync.dma_start(xl[s], xl_d[s])
            nc.sync.dma_start(cio, cio_d)
            nc.any.memset(ones3, 1.0)

            # PE p-state warm-up: the first real matmul otherwise runs at the
            # 0.65GHz cold state. Burn ~2us of dummy matmuls (ones3 x ones3,
            # no DMA dependency) into the second PSUM buffer during the input
            # DMA window; unit 1 overwrites it with start=True.
            pswarm = ppool.tile([P, UNIT], f32, tag="ps")
            for _ in range(48):
                nc.tensor.matmul(pswarm[:, 0:P], ones3, ones3[:, 0:P],
                                 start=True, stop=True)

            from collections import deque
            # (cost_ns, closure) DVE merge-ops dispensed into the MAX8->FIND
            # dependency gaps (~1.4us each): FIND(u) must wait out MAX8(u)'s
            # pipeline drain before it can read W; independent L2 work rides
            # in that window for free.
            pending = deque()

            def dispense(budget=1250):
                spent = 0
                while pending and spent + pending[0][0] <= budget:
                    cost, fn = pending.popleft()
                    fn()
                    spent += cost

            def emit_scans(t):
                qh_t = qh0[:, :] if t == 0 else qhr[:, (t - 1) * P:t * P]
                ql_t = ql0[:, :] if t == 0 else qlr[:, (t - 1) * P:t * P]
                W = wpool.tile([P, POOL], f32, tag="W")
                J16 = wpool.tile([P, POOL], u16, tag="J16")
                NS4 = UNIT // SEC
                for u in range(NU):
                    ps = ppool.tile([P, UNIT], f32, tag="ps")
                    base = u * UNIT
                    width = min(N - base, UNIT)
                    sw4 = [max(0, min(width - s * SEC, SEC)) for s in range(NS4)]
                    for lhs, rhs_of in ((qh_t, xh), (qh_t, xl), (ql_t, xh)):
                        first = rhs_of is xh and lhs is qh_t
                        for s in range(NS4):
                            w = sw4[s]
                            if w == 0:
                                continue
                            c0 = base + s * SEC
                            nc.tensor.matmul(ps[:, s * SEC:s * SEC + w], lhs,
                                             rhs_of[c0 // SW][:, c0 % SW:c0 % SW + w],
                                             start=first, stop=False)
                    for s in range(NS4):
                        w = sw4[s]
                        if w == 0:
                            continue
                        c0 = base + s * SEC
                        nc.tensor.matmul(ps[:, s * SEC:s * SEC + w], ones3,
                                         nb3[:, c0:c0 + w], start=False, stop=True)
                    nc.vector.max(out=W[:, u * 8:(u + 1) * 8], in_=ps[:, 0:width])
                    dispense(1500 if u == NU - 1 else 520)
                    nc.vector.max_index(out=J16[:, u * 8:(u + 1) * 8],
                                        in_max=W[:, u * 8:(u + 1) * 8],
                                        in_values=ps[:, 0:width])
                    dispense(520)
                return W, J16

            def queue_merge(t, W, J16):
                # C[p,i] = 65536*i + chunkbase_i + J16[p,i]  (fp32 exact, < 2^23)
                C = wpool.tile([P, POOL], f32, tag="C")
                V = wpool.tile([P, 24], f32, tag="V")
                Pu = wpool.tile([P, 24], u16, tag="Pu")
                Wb = wpool.tile([P, POOL], f32, tag="Wb")
                Wc = wpool.tile([P, POOL], f32, tag="Wc")
                Pu32 = wpool.tile([P, NSEL], f32, tag="Pu32")
                d3 = wpool.tile([P, NSEL * POOL], f32, tag="d3")
                d3v = d3[:, :].rearrange("p (s i) -> p s i", s=NSEL)
                G = wpool.tile([P, NSEL], f32, tag="G")
                Gi = wpool.tile([P, KOUT], mybir.dt.int32, tag="Gi")
                A = mybir.AluOpType
                F = mybir.ActivationFunctionType

                def act_tail_a():
                    # ACT: ranks 1..15 are final after L2 round 2 — start the
                    # |C - 65536*Pu_s| extraction early on the idle ACT queue.
                    nc.scalar.activation(out=Pu32[:, 0:15], in_=Pu[:, 1:16],
                                         func=F.Copy, scale=-65536.0)
                    for s in range(15):
                        nc.scalar.activation(out=d3[:, s * POOL:(s + 1) * POOL],
                                             in_=C, func=F.Abs,
                                             bias=Pu32[:, s:s + 1])

                def act_tail_b():
                    nc.scalar.activation(out=Pu32[:, 15:16], in_=Pu[:, 16:17],
                                         func=F.Copy, scale=-65536.0)
                    nc.scalar.activation(out=d3[:, 15 * POOL:16 * POOL],
                                         in_=C, func=F.Abs,
                                         bias=Pu32[:, 15:16])

                def fin():
                    nc.scalar.copy(out=Gi, in_=G)
                    nc.sync.dma_start(out_d[t * P:(t + 1) * P, :], Gi)

                ops = [
                    (260, lambda: nc.vector.tensor_tensor(out=C, in0=J16, in1=cio,
                                                          op=A.add)),
                    (260, lambda: nc.vector.max(out=V[:, 0:8], in_=W)),
                    (260, lambda: nc.vector.max_index(out=Pu[:, 0:8],
                                                      in_max=V[:, 0:8], in_values=W)),
                    (260, lambda: nc.vector.match_replace(out=Wb,
                                                          in_to_replace=V[:, 0:8],
                                                          in_values=W, imm_value=NEG)),
                    (260, lambda: nc.vector.max(out=V[:, 8:16], in_=Wb)),
                    (260, lambda: (nc.vector.max_index(out=Pu[:, 8:16],
                                                       in_max=V[:, 8:16],
                                                       in_values=Wb), act_tail_a())),
                    (260, lambda: nc.vector.match_replace(out=Wc,
                                                          in_to_replace=V[:, 8:16],
                                                          in_values=Wb,
                                                          imm_value=NEG)),
                    (260, lambda: nc.vector.max(out=V[:, 16:24], in_=Wc)),
                    (260, lambda: (nc.vector.max_index(out=Pu[:, 16:24],
                                                       in_max=V[:, 16:24],
                                                       in_values=Wc), act_tail_b())),
                    (1250, lambda: (nc.vector.tensor_reduce(
                        out=G, in_=d3v, axis=mybir.AxisListType.X,
                        op=A.min, apply_absolute_value=True), fin())),
                ]
                pending.extend(ops)

            hist = []
            for t in range(n_tiles):
                hist.append((t, emit_scans(t)))
                if len(hist) > 2:
                    tm, wj = hist.pop(0)
                    queue_merge(tm, *wj)
            for tm, wj in hist:
                queue_merge(tm, *wj)
            while pending:
                cost, fn = pending.popleft()
                fn()

    nc.compile()
    return nc


def _prep_inputs(x: np.ndarray):
    x = np.asarray(x, dtype=np.float32)
    xpad = np.zeros((NPAD, D), dtype=np.float32)
    xpad[:N] = x
    xT = xpad.T  # [D, NPAD]
    xhT = xT.astype(ml_dtypes.bfloat16)
    xlT = (xT - xhT.astype(np.float32)).astype(ml_dtypes.bfloat16)
    nb2 = np.full(NPAD, NEG, dtype=np.float32)
    nb2[:N] = (-0.5 * (x.astype(np.float64) ** 2).sum(1)).astype(np.float32)
    nb3 = np.ascontiguousarray(_split3_bf16(nb2))
    io = np.arange(POOL, dtype=np.float64)
    cio = np.broadcast_to(
        (io * 65536.0 + (io // 8) * UNIT).astype(np.float32), (P, POOL)).copy()
    NSEC = 8
    SW = NPAD // NSEC
    base = {"nb3": nb3, "cio": cio}
    for s in range(NSEC):
        base[f"xh{s}"] = np.ascontiguousarray(xhT[:, s * SW:(s + 1) * SW])
        base[f"xl{s}"] = np.ascontiguousarray(xlT[:, s * SW:(s + 1) * SW])
    in_maps = []
    for c in range(NCORES):
        r0 = c * RPC
        xq = np.zeros((NTILES * P, D), dtype=np.float32)
        end = min(r0 + NTILES * P, NPAD)
        xq[:end - r0] = xpad[r0:end]
        xqT = xq.T
        qh = xqT.astype(ml_dtypes.bfloat16)
        ql = (xqT - qh.astype(np.float32)).astype(ml_dtypes.bfloat16)
        m = dict(base)
        m["qh0"] = np.ascontiguousarray(qh[:, :P])
        m["ql0"] = np.ascontiguousarray(ql[:, :P])
        m["qhr"] = np.ascontiguousarray(qh[:, P:])
        m["qlr"] = np.ascontiguousarray(ql[:, P:])
        in_maps.append(m)
    return in_maps


def kernel(x, k):
    global _compiled
    assert int(k) == KOUT
    from concourse import bass_utils
    if _compiled is None:
        _compiled = build_program(NTILES)
    in_maps = _prep_inputs(x)
    out = np.empty((N, KOUT), dtype=np.int32)
    res = bass_utils.run_bass_kernel_spmd(_compiled, in_maps, core_ids=list(range(NCORES)))
    for c in range(NCORES):
        r0, r1 = c * RPC, (c + 1) * RPC
        out[r0:r1] = res.results[c]["out"][:r1 - r0]
    return out
